# revision 1
# baseline (speedup 1.0000x reference)
"""Trainium2 Bass kernel for nn_AttentionGRU (tree attention-GRU).

Self-contained: accepts FULL inputs, shards across 8 NeuronCores internally,
returns the FULL output (softmax probabilities, shape [4]).

Architecture
------------
Host (numpy):
  * compute dependency levels of the parent DAG (longest-path); reorder
    parents level-contiguously (each level padded to a multiple of 128 slots
    with duplicates of a real parent),
  * node blocks are striped over cores (block b -> core b%8) so every core
    owns 8 leaf blocks + 9 parent blocks,
  * pack the embedding table 4-rows-per-record (bf16, 512B records) so the
    record index fits int16 for the SWDGE dma_gather ucode; the 1-of-4
    select is folded into the per-(node,word) weight mask,
  * build wrapped-int16 index slabs (<=1024 indices per dma_gather call).

Device (SPMD on 8 cores, same program, per-core input slabs):
  1. Embedding: per local block, 4 dma_gather calls (1024 idxs each) fetch
     packed records; one DVE mask-multiply + one segmented reduce produce
     xe[128, 64]; PE-transpose into the bf16 xe^T slab.
  2. Leaf GRU on the core's own 8 leaf blocks (pre-collective); leaf h^T
     replaces leaf xe^T in the slab.  Two AllGathers (leaf h^T | parent
     xe^T) so the first overlaps the parent-half embedding work.
  3. Leaf h^T transposed to row layout -> node_h rows in DRAM (no GRU
     recompute post-collective).
  4. Level-batched scan: per <=16-block sub-batch one batched dma_gather of
     child rows, row-space attention-query matmuls straight into PSUM, DVE
     attention (sigmoid -> polynomial exp -> masked softmax -> h_tilde), GRU
     gates via bf16 PE matmuls in transposed layout (bias fused into the
     activation), h rows written back once per level.  Running max over
     parent h accumulated in transposed space.
  5. Output head + softmax (identical on every core; core 0's result used).
"""

import sys

sys.path.insert(0, "/opt/trn_rl_repo")
sys.path.insert(0, "/opt/trn_rl_repo/concourse")

import numpy as np

import concourse.bass as bass
import concourse.bacc as bacc
import concourse.tile as tile
from concourse import mybir
from concourse.masks import make_identity

F32 = mybir.dt.float32
BF16 = mybir.dt.bfloat16
I16 = mybir.dt.int16
ALU = mybir.AluOpType
AXL = mybir.AxisListType
ACT = mybir.ActivationFunctionType

NCORES = 8
H = 64          # hidden dim
K = 32          # words per node
D = 4           # max children
NCLASS = 4
SBMAX = 16      # scan sub-batch, in 128-blocks
GCALL = 1024    # max indices per dma_gather call (SWDGE ring capacity)

# minimax-ish degree-4 polynomial for exp(x) on [0,1] (Chebyshev interp)
_EXPC = np.polynomial.chebyshev.Chebyshev.interpolate(
    np.exp, 4, domain=[0, 1]).convert(kind=np.polynomial.Polynomial).coef


def _ap(base, dims):
    """Strided AP on the same tensor/partition-range as `base` (an AP),
    with explicit free dims [(step, count), ...] (steps in elements)."""
    return bass.AP(base.tensor, base.offset, [list(base.ap[0]), *[[s, c] for s, c in dims]])


def _wrap16(lin):
    """Linear int16 index list [n] (n%16==0) -> wrapped [128, n//16] layout
    read by the dma_gather ucode (index i at [i%16, i//16], replicated)."""
    seg = lin.reshape(-1, 16).T
    return np.tile(seg, (8, 1)).astype(np.int16)


def _split(nb, cap):
    nsb = -(-nb // cap)
    return [nb // nsb + (1 if i < nb % nsb else 0) for i in range(nsb)]


# ---------------------------------------------------------------------------
# host-side planning
# ---------------------------------------------------------------------------

def _plan(x_word, x_index, tree):
    N, Kw = x_index.shape
    P, D1 = tree.shape
    Dd = D1 - 1
    L = N - P
    children = tree[:, :Dd].astype(np.int64)

    # --- dependency levels -------------------------------------------------
    lvl = np.zeros(P, np.int64)
    for i in range(P):
        m = -1
        for c in children[i]:
            if c >= L and c - L < i:
                v = lvl[c - L]
                if v > m:
                    m = v
        lvl[i] = m + 1
    nlev = int(lvl.max()) + 1

    order = np.lexsort((np.arange(P), lvl))
    lv_sizes = np.bincount(lvl, minlength=nlev)

    # --- padded level layout ----------------------------------------------
    par_slots = []
    lv_blocks = []
    pos = 0
    for l in range(nlev):
        n = int(lv_sizes[l])
        real = order[pos:pos + n]
        pos += n
        nb = -(-n // 128)
        pad = nb * 128 - n
        par_slots.extend(real.tolist())
        par_slots.extend([int(real[0])] * pad)
        lv_blocks.append(nb)
    tot = L + len(par_slots)
    rem = (-tot) % (128 * NCORES)
    assert rem % 128 == 0
    if rem:
        par_slots.extend([par_slots[-1]] * rem)
        lv_blocks[-1] += rem // 128
    par_slots = np.asarray(par_slots, np.int64)
    NPS = len(par_slots)
    NEFF = L + NPS
    NBT = NEFF // 128
    NBC = NBT // NCORES
    LB = L // 128
    assert L % 128 == 0 and NBT % NCORES == 0
    LBC = LB // NCORES                      # leaf blocks per core
    assert LB % NCORES == 0

    first_slot = np.full(P, -1, np.int64)
    for s in range(NPS - 1, -1, -1):
        first_slot[par_slots[s]] = s

    slot_node = np.concatenate([np.arange(L), L + par_slots])

    def rows_of(slots):
        return ((slots % 128) * NBT + slots // 128).astype(np.int64)

    p_ar = np.arange(128)

    # --- embedding gather slabs (striped blocks; 1 block per chunk) --------
    # packed table record r = vocab rows [4r, 4r+4); idx16 = v//4, the 1-of-4
    # select is folded into mw (weight at sub-slot q=v%4, zero elsewhere).
    CPC = NBC * Kw                                   # columns per core
    gidx16 = np.zeros((NCORES, 128, CPC * 8), np.int16)
    mw = np.zeros((NCORES, 128, CPC * 4), np.float32)
    for c in range(NCORES):
        for m in range(NBC):
            b = c + NCORES * m
            nodes = slot_node[b * 128 + p_ar]
            xi = x_index[nodes, :]                   # [128, Kw]
            xw = x_word[nodes, :]
            for k in range(Kw):
                col = m * Kw + k
                mw[c, p_ar, col * 4 + xi[:, k] % 4] = xw[:, k]
            idx4 = (xi // 4).astype(np.int16)        # [128, Kw]
            for q in range(4):                       # gather call q: cols 8q..8q+8
                lin = idx4[:, 8 * q:8 * q + 8].T.reshape(-1)   # i=col*128+p
                gidx16[c, :, (m * 4 + q) * 64:(m * 4 + q + 1) * 64] = _wrap16(lin)

    # --- per-level child indices + masks ----------------------------------
    TOTB = sum(lv_blocks)
    TOTC = TOTB * D
    liall = np.zeros((128, TOTC), np.int64)
    lmall = np.zeros((128, TOTC), np.float32)
    zcall = np.zeros((128, TOTB), np.float32)
    pb0 = 0
    levels = []
    for l in range(nlev):
        nb = lv_blocks[l]
        for bl in range(nb):
            gb = pb0 + bl
            pi = par_slots[gb * 128 + p_ar]
            ch = children[pi]
            for d in range(Dd):
                cd = ch[:, d]
                valid = cd >= 0
                fwd = (cd >= L) & ((cd - L) >= pi)
                cslot = np.where(
                    cd < L, np.clip(cd, 0, None),
                    L + np.where(cd >= L, first_slot[np.clip(cd - L, 0, P - 1)], 0),
                )
                cslot = np.where(valid & ~fwd, cslot, 0)
                liall[:, gb * D + d] = rows_of(cslot)
                lmall[:, gb * D + d] = (valid & ~fwd).astype(np.float32)
                zcall[:, gb] += (valid & fwd) * np.float32(np.exp(0.5))
        levels.append(dict(nb=nb, pb0=pb0))
        pb0 += nb

    # scan gather calls: per level/sub-batch, <=1024-idx segments
    lidx_segs = []          # wrapped [128, n//16] blocks, concatenated later
    scan_calls = []         # per level: per sub-batch: list of (seg_col0, ncols, nidx)
    seg_col0 = 0
    for lv in levels:
        nb = lv["nb"]
        lv_calls = []
        sb0 = 0
        for sbn in _split(nb, SBMAX):
            c_lo = (lv["pb0"] + sb0) * D
            nc_s = sbn * D
            calls = []
            q0 = 0
            while q0 < nc_s:
                qn = min(8, nc_s - q0)               # columns this call
                nidx = qn * 128
                lin = liall[:, c_lo + q0:c_lo + q0 + qn].T.reshape(-1).astype(np.int16)
                lidx_segs.append(_wrap16(lin))
                calls.append((seg_col0, qn, nidx))
                seg_col0 += nidx // 16
                q0 += qn
            lv_calls.append(calls)
            sb0 += sbn
        scan_calls.append(lv_calls)
    lidx16 = np.concatenate(lidx_segs, axis=1)

    return dict(
        L=L, P=P, NEFF=NEFF, NBT=NBT, NBC=NBC, LB=LB, LBC=LBC, CPC=CPC, Kw=Kw,
        gidx16=gidx16, mw=mw, levels=levels, nlev=nlev,
        lidx16=lidx16, scan_calls=scan_calls,
        lmall=lmall, zcall=zcall, TOTB=TOTB, TOTC=TOTC,
    )


# ---------------------------------------------------------------------------
# device program
# ---------------------------------------------------------------------------

def _build(plan, vocab4):
    NBT, NBC, LB, LBC = plan["NBT"], plan["NBC"], plan["LB"], plan["LBC"]
    CPC, Kw = plan["CPC"], plan["Kw"]
    NEFF = plan["NEFF"]
    TOTB, TOTC = plan["TOTB"], plan["TOTC"]
    NLOC = NBC * 128
    NLEAF = LBC * 128                   # leaf columns per core (1024)
    NPARC = NLOC - NLEAF                # parent columns per core (1152)
    LIDXW = plan["lidx16"].shape[1]
    levels = plan["levels"]
    scan_calls = plan["scan_calls"]

    nc = bacc.Bacc(None, num_devices=NCORES)

    e4 = nc.dram_tensor("e4", [vocab4, 4 * H], BF16, kind="ExternalInput")
    gidx = nc.dram_tensor("gidx", [128, CPC * 8], I16, kind="ExternalInput")
    mw_e = nc.dram_tensor("mw_e", [128, CPC * 4], BF16, kind="ExternalInput")
    wz_e = nc.dram_tensor("wz_e", [H, H], BF16, kind="ExternalInput")
    wr_e = nc.dram_tensor("wr_e", [H, H], BF16, kind="ExternalInput")
    wh_e = nc.dram_tensor("wh_e", [H, H], BF16, kind="ExternalInput")
    wa_e = nc.dram_tensor("wa_e", [H, H], BF16, kind="ExternalInput")
    uz_t = nc.dram_tensor("uz_t", [H, H], BF16, kind="ExternalInput")
    ur_t = nc.dram_tensor("ur_t", [H, H], BF16, kind="ExternalInput")
    uh_t = nc.dram_tensor("uh_t", [H, H], BF16, kind="ExternalInput")
    bias_e = nc.dram_tensor("bias_e", [H, 3], F32, kind="ExternalInput")
    wo_e = nc.dram_tensor("wo_e", [H + 1, NCLASS], F32, kind="ExternalInput")
    li_e = nc.dram_tensor("li_e", [128, LIDXW], I16, kind="ExternalInput")
    lm_e = nc.dram_tensor("lm_e", [128, TOTC], F32, kind="ExternalInput")
    zc_e = nc.dram_tensor("zc_e", [128, TOTB], F32, kind="ExternalInput")

    out_t = nc.dram_tensor("out", [1, NCLASS], F32, kind="ExternalOutput")

    # node_h rows stored bf16, duplicated ([h, h] = 256B records) so the
    # dma_gather 256B-min element covers exactly one node
    node_h = nc.dram_tensor("node_h", [NEFF, 2 * H], BF16)               # Internal
    # leaf h travels ROW-major (owner pre-transposes its 8 blocks)
    agL_in = nc.dram_tensor("agL_in", [128, LBC * H], BF16)              # Internal
    agP_in = nc.dram_tensor("agP_in", [H, NPARC], BF16)                  # Internal
    agL_out = nc.dram_tensor("agL_out", [NCORES, 128, LBC * H], BF16,
                             addr_space="Shared")
    agP_out = nc.dram_tensor("agP_out", [NCORES, H, NPARC], BF16, addr_space="Shared")

    node_h3 = node_h[:].rearrange("(p b) e -> p b e", p=128)             # [128,NBT,128]

    with tile.TileContext(nc) as tc:
        with (
            tc.tile_pool(name="const", bufs=1) as constp,
            tc.tile_pool(name="small", bufs=2) as small,
            tc.tile_pool(name="lvl", bufs=2) as lvlp,
            tc.tile_pool(name="ph1", bufs=2) as ph1p,
            tc.tile_pool(name="scan", bufs=2) as scanp,
            tc.tile_pool(name="psum", bufs=2, space="PSUM") as psum,
            tc.tile_pool(name="psumt", bufs=2, space="PSUM") as psumt,
            tc.tile_pool(name="psumq", bufs=1, space="PSUM") as psumq,
        ):
            ident = constp.tile([128, 128], F32, tag="ident")
            make_identity(nc, ident[:])
            identb = constp.tile([128, 128], BF16, tag="identb")
            nc.vector.tensor_copy(out=identb[:], in_=ident[:])

            wts = {}
            for nm, t in [("wz", wz_e), ("wr", wr_e), ("wh", wh_e), ("wa", wa_e),
                          ("uz", uz_t), ("ur", ur_t), ("uh", uh_t)]:
                w = constp.tile([H, H], BF16, tag=f"w_{nm}")
                nc.sync.dma_start(out=w[:], in_=t[:])
                wts[nm] = w
            bias = constp.tile([H, 3], F32, tag="bias")
            nc.sync.dma_start(out=bias[:], in_=bias_e[:])
            bz, br, bh = bias[:, 0:1], bias[:, 1:2], bias[:, 2:3]
            wo = constp.tile([H + 1, NCLASS], F32, tag="w_wo")
            nc.sync.dma_start(out=wo[:], in_=wo_e[:])

            liall = constp.tile([128, LIDXW], I16, tag="liall")
            nc.sync.dma_start(out=liall[:], in_=li_e[:])
            lmall = constp.tile([128, TOTC], F32, tag="lmall")
            nc.sync.dma_start(out=lmall[:], in_=lm_e[:])
            zcall = constp.tile([128, TOTB], F32, tag="zcall")
            nc.sync.dma_start(out=zcall[:], in_=zc_e[:])

            gidx_t = constp.tile([128, CPC * 8], I16, tag="gidx")
            nc.sync.dma_start(out=gidx_t[:], in_=gidx[:])
            mw_t = constp.tile([128, CPC * 4], BF16, tag="mw")
            nc.sync.dma_start(out=mw_t[:], in_=mw_e[:])

            xeT_loc = constp.tile([H, NLOC], BF16, tag="xeT_loc")

            # zero-init parent region of node_h (scan gathers read the whole
            # tensor; fwd-ref/masked columns must see finite data)
            zt = constp.tile([128, 9 * 2 * H], BF16, tag="zeros")
            nc.vector.memset(zt[:], 0.0)
            zb0 = LB
            while zb0 < NBT:
                zn = min(9, NBT - zb0)
                nc.sync.dma_start(out=node_h3[:, zb0:zb0 + zn, :],
                                  in_=zt[:, :zn * 2 * H])
                zb0 += zn

            # shared num_idxs registers for the dma_gather calls
            greg = {}

            def nreg(n):
                if n not in greg:
                    greg[n] = nc.gpsimd.to_reg(n)
                return greg[n]

            # ---------------- phase 1: embedding ---------------------------
            def emb_chunk(m):
                """local block m: gather packed records, combine, transpose."""
                g = ph1p.tile([128, Kw * 4 * H], BF16, tag="ph1")
                for q in range(4):
                    nc.gpsimd.dma_gather(
                        out_ap=g[:, q * 8 * 4 * H:(q + 1) * 8 * 4 * H].rearrange(
                            "p (j e) -> p j e", e=4 * H),
                        in_ap=e4[:],
                        idxs_ap=gidx_t[:, (m * 4 + q) * 64:(m * 4 + q + 1) * 64],
                        num_idxs=GCALL, num_idxs_reg=nreg(GCALL), elem_size=4 * H)
                # select+weight: g *= mw (broadcast over the 64 row elements);
                # alternate the multiply between DVE and Pool to balance load
                gk = g[:].rearrange("p (kq e) -> p kq e", e=H)
                eng = nc.gpsimd if m % 2 == 1 else nc.vector
                eng.tensor_tensor(
                    out=gk, in0=gk,
                    in1=_ap(mw_t[:, m * Kw * 4:(m + 1) * Kw * 4], [(1, Kw * 4), (0, H)]),
                    op=ALU.mult)
                xe_c = small.tile([128, H], F32, tag="xe_c")
                nc.vector.tensor_reduce(
                    out=xe_c[:], in_=_ap(g[:], [(1, H), (H, Kw * 4)]),
                    axis=AXL.X, op=ALU.add)
                pt = psumt.tile([H, 128], F32, tag="pt")
                nc.tensor.transpose(out=pt[:], in_=xe_c[:], identity=ident[:])
                nc.scalar.activation(
                    out=xeT_loc[:, m * 128:(m + 1) * 128], in_=pt[:], func=ACT.Copy)

            for m in range(LBC):                    # leaf blocks first
                emb_chunk(m)

            # ---------------- phase 2a: own-leaf GRU (pre-collective) ------
            lrow = constp.tile([128, LBC * H], BF16, tag="lrow")
            for c0 in range(0, NLEAF, 512):
                sl = slice(c0, c0 + 512)
                pz = psum.tile([H, 512], F32, tag="pz")
                nc.tensor.matmul(out=pz[:], lhsT=wts["wz"][:], rhs=xeT_loc[:, sl],
                                 start=True, stop=True)
                zl = small.tile([H, 512], F32, tag="zt_")
                nc.scalar.activation(out=zl[:], in_=pz[:], func=ACT.Sigmoid, bias=bz)
                ph = psum.tile([H, 512], F32, tag="pz")
                nc.tensor.matmul(out=ph[:], lhsT=wts["wh"][:], rhs=xeT_loc[:, sl],
                                 start=True, stop=True)
                cl = small.tile([H, 512], F32, tag="ct_")
                nc.scalar.activation(out=cl[:], in_=ph[:], func=ACT.Tanh, bias=bh)
                nc.vector.tensor_tensor(out=zl[:], in0=zl[:], in1=cl[:], op=ALU.mult)
                nc.vector.tensor_tensor(out=cl[:], in0=cl[:], in1=zl[:], op=ALU.subtract)
                # transpose own leaf h to ROW layout for the collective
                for gr in range(4):
                    pt = psumt.tile([128, H], F32, tag="pt")
                    nc.tensor.transpose(
                        out=pt[:], in_=cl[:, gr * 128:(gr + 1) * 128],
                        identity=ident[:H, :H])
                    nc.scalar.activation(
                        out=lrow[:, (c0 // 128 + gr) * H:(c0 // 128 + gr + 1) * H],
                        in_=pt[:], func=ACT.Copy)

            nc.gpsimd.dma_start(out=agL_in[:], in_=lrow[:])
            nc.gpsimd.collective_compute(
                "AllGather", ALU.bypass,
                replica_groups=[list(range(NCORES))],
                ins=[agL_in[:].opt()], outs=[agL_out[:].opt()])

            for m in range(LBC, NBC):               # parent blocks
                emb_chunk(m)

            nc.gpsimd.dma_start(out=agP_in[:], in_=xeT_loc[:, NLEAF:])
            nc.gpsimd.collective_compute(
                "AllGather", ALU.bypass,
                replica_groups=[list(range(NCORES))],
                ins=[agP_in[:].opt()], outs=[agP_out[:].opt()])

            # ---------------- phase 2b: leaf rows to node_h ----------------
            # the collective already carries rows; just write the dup halves.
            # Emitted BEFORE the xeT assembly: the SP DMA queue is in-order,
            # and these only wait on collective-L, so they run under the
            # parent-half embedding instead of stalling behind collective-P.
            for c in range(NCORES):
                for hf in range(2):
                    nc.sync.dma_start(
                        out=_ap(node_h3[:, c:c + 1, hf * H:hf * H + 1],
                                [(NCORES * 2 * H, LBC), (1, H)]),
                        in_=agL_out[c].rearrange("p (j e) -> p j e", e=H))

            # parent xe^T assembly: core c's slab covers parent blocks
            # 64 + (c + 8j'), j'=0..8 -> xeT cols c*128 + j'*1024
            xeT = constp.tile([H, TOTB * 128], BF16, tag="xeT")
            for c in range(NCORES):
                nc.sync.dma_start(
                    out=_ap(xeT[:, c * 128:c * 128 + 1],
                            [(NCORES * 128, NBC - LBC), (1, 128)]),
                    in_=agP_out[c].rearrange("e (j x) -> e j x", x=128))

            finacc = constp.tile([H, 1], F32, tag="finacc")
            nc.vector.memset(finacc[:], -1e30)
            # output-head constants prepared off the critical tail
            fin = small.tile([H + 1, 1], F32, tag="fin")
            nc.vector.memset(fin[H:H + 1, :], 1.0)

            # ---------------- phase 3: level scan (sub-batched) ------------
            sg_seen = 0

            def sub_gather(l, si):
                """Emit the child-row gather calls for (level, sub-batch)."""
                nonlocal sg_seen
                g = scanp.tile([128, SBMAX * D * 2 * H], BF16, tag="sg")
                if sg_seen < 2:
                    # make stale SBUF finite for masked-out columns (the
                    # mask zeroes their contribution, but NaN*0 != 0)
                    nc.vector.memset(g[:], 0.0)
                    sg_seen += 1
                col = 0
                for (seg0, qn, nidx) in scan_calls[l][si]:
                    nc.gpsimd.dma_gather(
                        out_ap=g[:, col * 2 * H:(col + qn) * 2 * H].rearrange(
                            "p (j e) -> p j e", e=2 * H),
                        in_ap=node_h[:],
                        idxs_ap=liall[:, seg0:seg0 + nidx // 16],
                        num_idxs=nidx, num_idxs_reg=nreg(nidx), elem_size=2 * H)
                    col += qn
                return g

            # level-0 gathers depend only on leaf rows: issue them now so
            # they run under the parent-half collective
            l0_g = [sub_gather(0, si) for si in range(len(scan_calls[0]))]

            for l, lv in enumerate(levels):
                nb = lv["nb"]
                last_lv = l == len(levels) - 1
                # the last level's h rows are never gathered (the head reads
                # finacc); skip its row transposes and node_h write entirely
                hrow_all = None if last_lv else lvlp.tile(
                    [128, nb * H], BF16, tag="hrow")
                # all of this level's gathers upfront: they only depend on
                # earlier levels' writes, and emitting them before any of
                # this level's writes avoids false whole-tensor deps
                gs = l0_g if l == 0 else [
                    sub_gather(l, si) for si in range(len(scan_calls[l]))]
                # software-pipelined emission: all attention front-ends
                # first, then all gate/write back-ends — sub-batches within
                # a level are independent, and this keeps sb2's sigmoid and
                # DVE chain from queueing behind sb1's back-end
                fes = []
                sb0 = 0
                for si, sbn in enumerate(_split(nb, SBMAX)):
                    n_s = sbn * 128
                    nc_s = sbn * D
                    gb0 = lv["pb0"] + sb0
                    c_lo = gb0 * D
                    rhs_s = xeT[:, gb0 * 128:gb0 * 128 + n_s]

                    g = gs[si]
                    # child h = first half of each 256B dup record
                    g4 = _ap(g[:, 0:1], [(D * 2 * H, sbn), (2 * H, D), (1, H)])

                    # attention query rows straight into PSUM
                    qps = psumq.tile([128, SBMAX * H], F32, tag="qps")
                    for b in range(sbn):
                        nc.tensor.matmul(
                            out=qps[:, b * H:(b + 1) * H],
                            lhsT=rhs_s[:, b * 128:(b + 1) * 128],
                            rhs=wts["wa"][:], start=True, stop=True)

                    # logits = sum_e q*child
                    prod = scanp.tile([128, SBMAX * D * H], BF16, tag="sp")
                    pv = prod[:, :nc_s * H]
                    q_ap = _ap(qps[:, :sbn * H], [(H, sbn), (0, D), (1, H)])
                    nc.vector.tensor_tensor(
                        out=pv.rearrange("p (b d e) -> p b d e", b=sbn, d=D),
                        in0=g4, in1=q_ap, op=ALU.mult)
                    logit = lvlp.tile([128, SBMAX * D], F32, tag="logit")
                    nc.vector.tensor_reduce(
                        out=logit[:, :nc_s],
                        in_=pv.rearrange("p (c e) -> p c e", e=H),
                        axis=AXL.X, op=ALU.add)
                    lg = logit[:, :nc_s]
                    nc.scalar.activation(out=lg, in_=lg, func=ACT.Sigmoid)
                    # exp(s) on [0,1] via Horner (keeps ACT on the sigmoid set)
                    ex = lvlp.tile([128, SBMAX * D], F32, tag="ex")
                    ev = ex[:, :nc_s]
                    nc.vector.tensor_scalar(out=ev, in0=lg, scalar1=float(_EXPC[4]),
                                            scalar2=float(_EXPC[3]), op0=ALU.mult,
                                            op1=ALU.add)
                    for cf in (_EXPC[2], _EXPC[1]):
                        nc.vector.tensor_tensor(out=ev, in0=ev, in1=lg, op=ALU.mult)
                        nc.vector.tensor_scalar_add(out=ev, in0=ev, scalar1=float(cf))
                    nc.vector.tensor_tensor(out=ev, in0=ev, in1=lg, op=ALU.mult)
                    # (ev + c0) * mask in one fused op
                    nc.vector.scalar_tensor_tensor(
                        out=ev, in0=ev, scalar=float(_EXPC[0]),
                        in1=lmall[:, c_lo:c_lo + nc_s],
                        op0=ALU.add, op1=ALU.mult)
                    den = lvlp.tile([128, SBMAX], F32, tag="den")
                    nc.vector.tensor_reduce(
                        out=den[:, :sbn],
                        in_=ev.rearrange("p (b d) -> p b d", d=D),
                        axis=AXL.X, op=ALU.add)
                    if plan["zcall"][:, gb0:gb0 + sbn].any():
                        # fwd-ref denominator correction; skipped when the
                        # host knows this range has no forward references
                        nc.vector.tensor_tensor(
                            out=den[:, :sbn], in0=den[:, :sbn],
                            in1=zcall[:, gb0:gb0 + sbn], op=ALU.add)
                    nc.vector.reciprocal(out=den[:, :sbn], in_=den[:, :sbn])
                    nc.vector.tensor_tensor(
                        out=ev.rearrange("p (b d) -> p b d", d=D),
                        in0=ev.rearrange("p (b d) -> p b d", d=D),
                        in1=_ap(den[:, :sbn], [(1, sbn), (0, D)]), op=ALU.mult)
                    # h_tilde = sum_d attn*child
                    a_ap = _ap(ev, [(D, sbn), (1, D), (0, H)])
                    nc.vector.tensor_tensor(
                        out=pv.rearrange("p (b d e) -> p b d e", b=sbn, d=D),
                        in0=g4, in1=a_ap, op=ALU.mult)
                    ht = lvlp.tile([128, SBMAX * H], F32, tag="ht")
                    nc.vector.tensor_reduce(
                        out=ht[:, :sbn * H],
                        in_=_ap(pv, [(D * H, sbn), (1, H), (H, D)]),
                        axis=AXL.X, op=ALU.add)
                    fes.append((sbn, sb0, n_s, rhs_s, ht))
                    sb0 += sbn

                for (sbn, sb0, n_s, rhs_s, ht) in fes:
                    # transpose h_tilde -> [64, n_s] (bf16 for the U matmuls)
                    htT = lvlp.tile([H, SBMAX * 128], BF16, tag="htT")
                    for b in range(sbn):
                        pt = psumt.tile([H, 128], F32, tag="pt")
                        nc.tensor.transpose(out=pt[:], in_=ht[:, b * H:(b + 1) * H],
                                            identity=ident[:])
                        nc.scalar.activation(out=htT[:, b * 128:(b + 1) * 128],
                                             in_=pt[:], func=ACT.Copy)

                    # gates; weight-grouped matmul passes (each lhsT loaded
                    # once per sub-batch), one 4-bank PSUM tile per gate,
                    # full-width activations and combines
                    hTt = lvlp.tile([H, SBMAX * 128], BF16, tag="hTt")
                    rh = lvlp.tile([H, SBMAX * 128], BF16, tag="rh")
                    chunks = [(c0, min(512, n_s - c0))
                              for c0 in range(0, n_s, 512)]

                    def gate_mm(wname, uname, urhs, act, bias_ap, dst):
                        # half-width passes: each weight loaded once per half,
                        # two PSUM buffers keep the gate phases pipelined
                        for h0 in range(0, n_s, 1024):
                            hw = min(1024, n_s - h0)
                            hch = [(c0, cw) for (c0, cw) in chunks
                                   if h0 <= c0 < h0 + hw]
                            p = psum.tile([H, 1024], F32, tag="pz")
                            for (c0, cw) in hch:
                                nc.tensor.matmul(out=p[:, c0 - h0:c0 - h0 + cw],
                                                 lhsT=wts[wname][:],
                                                 rhs=rhs_s[:, c0:c0 + cw],
                                                 start=True, stop=False)
                            for (c0, cw) in hch:
                                nc.tensor.matmul(out=p[:, c0 - h0:c0 - h0 + cw],
                                                 lhsT=wts[uname][:],
                                                 rhs=urhs[:, c0:c0 + cw],
                                                 start=False, stop=True)
                            nc.scalar.activation(out=dst[:, h0:h0 + hw],
                                                 in_=p[:, :hw],
                                                 func=act, bias=bias_ap)

                    # r first: its act -> rh -> c-U chain is the serial tail,
                    # so give it a head start while z fills the PE queue
                    rT = lvlp.tile([H, SBMAX * 128], BF16, tag="rT")
                    gate_mm("wr", "ur", htT, ACT.Sigmoid, br, rT)
                    nc.vector.tensor_tensor(out=rh[:, :n_s], in0=rT[:, :n_s],
                                            in1=htT[:, :n_s], op=ALU.mult)
                    zT = lvlp.tile([H, SBMAX * 128], BF16, tag="zT")
                    gate_mm("wz", "uz", htT, ACT.Sigmoid, bz, zT)
                    cT = lvlp.tile([H, SBMAX * 128], BF16, tag="cT")
                    gate_mm("wh", "uh", rh, ACT.Tanh, bh, cT)
                    # h = c + z*(ht - c)
                    nc.vector.tensor_tensor(out=hTt[:, :n_s], in0=htT[:, :n_s],
                                            in1=cT[:, :n_s], op=ALU.subtract)
                    nc.vector.tensor_tensor(out=hTt[:, :n_s], in0=hTt[:, :n_s],
                                            in1=zT[:, :n_s], op=ALU.mult)
                    nc.vector.tensor_tensor(out=hTt[:, :n_s], in0=hTt[:, :n_s],
                                            in1=cT[:, :n_s], op=ALU.add)
                    # transpose back into the level-wide row buffer, then
                    # write this sub-batch's rows immediately (overlaps the
                    # next sub-batch's compute; safe because all of this
                    # level's gathers were emitted before any write)
                    if not last_lv:
                        for b in range(sbn):
                            pt = psumt.tile([128, H], BF16, tag="pt")
                            nc.tensor.transpose(out=pt[:],
                                                in_=hTt[:, b * 128:(b + 1) * 128],
                                                identity=identb[:H, :H])
                            nc.scalar.activation(
                                out=hrow_all[:, (sb0 + b) * H:(sb0 + b + 1) * H],
                                in_=pt[:], func=ACT.Copy)
                        bb = LB + lv["pb0"] + sb0
                        for hf in range(2):
                            nc.sync.dma_start(
                                out=_ap(node_h3[:, bb:bb + 1, hf * H:hf * H + 1],
                                        [(2 * H, sbn), (1, H)]),
                                in_=_ap(hrow_all[:, sb0 * H:sb0 * H + 1],
                                        [(H, sbn), (1, H)]))
                    # fold this sub-batch into the running parent-h max
                    lmax = lvlp.tile([H, 1], F32, tag="lmax")
                    nc.vector.tensor_reduce(out=lmax[:], in_=hTt[:, :n_s],
                                            axis=AXL.X, op=ALU.max)
                    nc.vector.tensor_tensor(out=finacc[:], in0=finacc[:],
                                            in1=lmax[:], op=ALU.max)

            # prefetch the Exp activation table while the tail of the scan
            # still runs (the head's softmax otherwise pays the swap serially)
            nc.scalar.activation(out=fin[H:H + 1, :], in_=fin[H:H + 1, :],
                                 func=ACT.Exp)
            nc.vector.memset(fin[H:H + 1, :], 1.0)

            # ---------------- phase 4: output head -------------------------
            nc.vector.tensor_copy(out=fin[:H, :], in_=finacc[:])
            po = psumt.tile([NCLASS, 1], F32, tag="pt")
            nc.tensor.matmul(out=po[:], lhsT=wo[:], rhs=fin[:], start=True, stop=True)
            s4 = small.tile([NCLASS, 1], F32, tag="s4")
            nc.vector.tensor_copy(out=s4[:], in_=po[:])
            s4t = small.tile([1, NCLASS], F32, tag="s4t")
            nc.sync.dma_start(out=s4t[:], in_=s4[:])
            nc.scalar.activation(out=s4t[:], in_=s4t[:], func=ACT.Exp)
            ssum = small.tile([1, 1], F32, tag="ssum")
            nc.vector.tensor_reduce(out=ssum[:], in_=s4t[:], axis=AXL.X, op=ALU.add)
            nc.vector.reciprocal(out=ssum[:], in_=ssum[:])
            nc.vector.tensor_tensor(out=s4t[:], in0=s4t[:],
                                    in1=_ap(ssum[:], [(0, NCLASS)]), op=ALU.mult)
            nc.sync.dma_start(out=out_t[:], in_=s4t[:])

    nc.compile()
    return nc


# ---------------------------------------------------------------------------
# entry point
# ---------------------------------------------------------------------------

def _to_bf16(x):
    import ml_dtypes
    return np.asarray(x, np.float32).astype(ml_dtypes.bfloat16)


def _prepare(inputs):
    x_word = np.asarray(inputs["x_word"], np.float32)
    x_index = np.asarray(inputs["x_index"], np.int32)
    tree = np.asarray(inputs["tree"], np.int32)
    E = np.asarray(inputs["E_bu"], np.float32)
    vocab = E.shape[1]
    assert vocab % 4 == 0
    vocab4 = vocab // 4

    plan = _plan(x_word, x_index, tree)
    nc = _build(plan, vocab4)

    e4 = np.ascontiguousarray(E.T).reshape(vocab4, 4 * H)

    bias3 = np.stack([
        np.asarray(inputs["b_z_bu"], np.float32).reshape(-1),
        np.asarray(inputs["b_r_bu"], np.float32).reshape(-1),
        np.asarray(inputs["b_h_bu"], np.float32).reshape(-1),
    ], axis=1)

    shared = {
        "e4": _to_bf16(e4),
        "wz_e": _to_bf16(np.asarray(inputs["W_z_bu"]).T),
        "wr_e": _to_bf16(np.asarray(inputs["W_r_bu"]).T),
        "wh_e": _to_bf16(np.asarray(inputs["W_h_bu"]).T),
        "wa_e": _to_bf16(inputs["W_attn"]),
        "uz_t": _to_bf16(np.asarray(inputs["U_z_bu"]).T),
        "ur_t": _to_bf16(np.asarray(inputs["U_r_bu"]).T),
        "uh_t": _to_bf16(np.asarray(inputs["U_h_bu"]).T),
        "bias_e": bias3,
        "wo_e": np.vstack([np.asarray(inputs["W_out_bu"], np.float32).T,
                           np.asarray(inputs["b_out_bu"], np.float32).reshape(1, -1)]),
        "li_e": plan["lidx16"],
        "lm_e": plan["lmall"],
        "zc_e": plan["zcall"],
    }

    in_maps = []
    for c in range(NCORES):
        m = dict(shared)
        m["gidx"] = plan["gidx16"][c]
        m["mw_e"] = _to_bf16(plan["mw"][c])
        in_maps.append(m)
    return nc, in_maps


def kernel(**inputs) -> np.ndarray:
    from concourse.bass_utils import run_bass_kernel_spmd
    nc, in_maps = _prepare(inputs)
    res = run_bass_kernel_spmd(nc, in_maps, core_ids=list(range(NCORES)))
    return res.results[0]["out"].reshape(NCLASS).astype(np.float32)



# revision 59
# speedup vs baseline: 7.4199x; 7.4199x over previous
"""Trainium2 Bass kernel for nn_AttentionGRU (tree attention-GRU).

Self-contained: accepts FULL inputs, shards across 8 NeuronCores internally,
returns the FULL output (softmax probabilities, shape [4]).

Architecture (v2)
-----------------
Host (numpy):
  * compute dependency levels of the parent DAG (longest-path); reorder
    parents level-contiguously (levels padded to 128-blocks; the global
    alignment padding goes to level 0, keeping the tail levels minimal),
  * node blocks are striped over cores (block b -> core b%8),
  * embedding table kept as f32 rows (256B records, exactly the SWDGE
    minimum element).  The vocab is split in 4 quarters of 25000 so the
    record index fits int16; per (block, quarter) the host packs each
    partition's (deduped) words into a shared column grid (ragged,
    per-block sized), pads with idx 0 / weight 0,
  * per-block per-quarter gather calls (<=4096 idx each, ring enlarged).

Device (SPMD on 8 cores, same program, per-core input slabs):
  1. Embedding: per local block, 4 dma_gather calls fetch f32 rows; one
     DVE broadcast-multiply by the slot weights (-> bf16) and a bf16
     2x-mode tree reduction produce xe[128, 64]; PE-transpose into the
     bf16 xe^T slab.
  2. Leaf GRU on the core's own 8 leaf blocks; leaf h^T -> rows.
     Three AllGathers: leaf h rows, then parent xe^T in two halves so
     the first half (which covers all of level 0) lands while the
     second half's embedding work and collective overlap the scan start.
  3. Level-batched scan: per <=16-block sub-batch one batched dma_gather
     of child rows, attention-query matmuls into PSUM (copied to SBUF
     bf16), 2x-mode DVE attention (mult + bf16 tree reductions, sigmoid
     -> 5-op Horner exp -> masked softmax -> h_tilde), GRU gates via bf16
     PE matmuls in transposed layout, duplicated h rows written with a
     single DMA per sub-batch.  Running parent-h max kept as a full-width
     bf16 tensor_tensor(max), reduced once at the end.
  4. Output head + softmax (identical on every core; core 0's result used).
"""

import sys

sys.path.insert(0, "/opt/trn_rl_repo")
sys.path.insert(0, "/opt/trn_rl_repo/concourse")

import numpy as np

import concourse.bass as bass
import concourse.bacc as bacc
import concourse.tile as tile
from concourse import mybir
from concourse.masks import make_identity

F32 = mybir.dt.float32
BF16 = mybir.dt.bfloat16
I16 = mybir.dt.int16
ALU = mybir.AluOpType
AXL = mybir.AxisListType
ACT = mybir.ActivationFunctionType

NCORES = 8
H = 64          # hidden dim
K = 32          # words per node
D = 4           # max children
NCLASS = 4
SBMAX = 12      # scan sub-batch, in 128-blocks
QN = 4          # vocab quarters (int16 record index limit)
GCALL = 1024    # max indices per dma_gather call (SWDGE ring capacity)
SCRATCH = 16384  # dynamic dma scratch (default 1024-descriptor ring)

# minimax-ish degree-4 polynomial for exp(x) on [0,1] (Chebyshev interp)
_EXPC = np.polynomial.chebyshev.Chebyshev.interpolate(
    np.exp, 4, domain=[0, 1]).convert(kind=np.polynomial.Polynomial).coef


def _ap(base, dims):
    """Strided AP on the same tensor/partition-range as `base` (an AP),
    with explicit free dims [(step, count), ...] (steps in elements)."""
    return bass.AP(base.tensor, base.offset, [list(base.ap[0]), *[[s, c] for s, c in dims]])


def _wrap16(lin):
    """Linear int16 index list [n] (n%16==0) -> wrapped [128, n//16] layout
    read by the dma_gather ucode (index i at [i%16, i//16], replicated)."""
    seg = lin.reshape(-1, 16).T
    return np.tile(seg, (8, 1)).astype(np.int16)


def _split(nb, cap):
    nsb = -(-nb // cap)
    return [nb // nsb + (1 if i < nb % nsb else 0) for i in range(nsb)]


# ---------------------------------------------------------------------------
# host-side planning
# ---------------------------------------------------------------------------

def _plan(x_word, x_index, tree, vocab):
    N, Kw = x_index.shape
    P, D1 = tree.shape
    Dd = D1 - 1
    L = N - P
    children = tree[:, :Dd].astype(np.int64)
    assert vocab % QN == 0
    QSZ = vocab // QN
    assert QSZ - 1 <= 32767

    # --- dependency levels -------------------------------------------------
    lvl = np.zeros(P, np.int64)
    for i in range(P):
        m = -1
        for c in children[i]:
            if c >= L and c - L < i:
                v = lvl[c - L]
                if v > m:
                    m = v
        lvl[i] = m + 1
    nlev = int(lvl.max()) + 1

    order = np.lexsort((np.arange(P), lvl))
    lv_sizes = np.bincount(lvl, minlength=nlev)

    # --- padded level layout (alignment padding goes to level 0) -----------
    nbs = [int(-(-int(lv_sizes[l]) // 128)) for l in range(nlev)]
    rem = (-(L + 128 * sum(nbs))) % (128 * NCORES)
    assert rem % 128 == 0
    nbs[0] += rem // 128

    par_slots = []
    pos = 0
    for l in range(nlev):
        n = int(lv_sizes[l])
        real = order[pos:pos + n]
        pos += n
        pad = nbs[l] * 128 - n
        par_slots.extend(real.tolist())
        par_slots.extend([int(real[0])] * pad)
    lv_blocks = nbs
    par_slots = np.asarray(par_slots, np.int64)
    NPS = len(par_slots)
    NEFF = L + NPS
    NBT = NEFF // 128
    NBC = NBT // NCORES
    LB = L // 128
    assert L % 128 == 0 and NBT % NCORES == 0
    LBC = LB // NCORES                      # leaf blocks per core
    assert LB % NCORES == 0

    first_slot = np.full(P, -1, np.int64)
    for s in range(NPS - 1, -1, -1):
        first_slot[par_slots[s]] = s

    slot_node = np.concatenate([np.arange(L), L + par_slots])

    def rows_of(slots):
        return ((slots % 128) * NBT + slots // 128).astype(np.int64)

    p_ar = np.arange(128)

    # --- 4-row-packed embedding slabs (512B records, idx = v//4 < 25000) ---
    vocab4 = vocab // 4
    gidx4 = np.zeros((NCORES, 128, NBC * Kw * 8), np.int16)
    mw4 = np.zeros((NCORES, 128, NBC * Kw * 4), np.float32)
    for bi in range(NBC):
        m = bi
        for c in range(NCORES):
            b = c + NCORES * m
            nodes = slot_node[b * 128 + p_ar]
            xi = x_index[nodes, :].astype(np.int64)
            xw = x_word[nodes, :].astype(np.float32)
            for k in range(Kw):
                mw4[c, p_ar, (bi * Kw + k) * 4 + xi[:, k] % 4] = xw[:, k]
            idx4 = (xi // 4).astype(np.int16)            # [128, Kw]
            lin = idx4.T.reshape(-1)                     # i = k*128 + p
            gidx4[c, :, bi * Kw * 8:(bi + 1) * Kw * 8] = _wrap16(lin)
    assert vocab4 - 1 <= 32767

    # --- per-level child indices + masks ----------------------------------
    TOTB = sum(lv_blocks)
    TOTC = TOTB * D
    liall = np.zeros((128, TOTC), np.int64)
    lmall = np.zeros((128, TOTC), np.float32)
    zcall = np.zeros((128, TOTB), np.float32)
    pb0 = 0
    levels = []
    for l in range(nlev):
        nb = lv_blocks[l]
        for bl in range(nb):
            gb = pb0 + bl
            pi = par_slots[gb * 128 + p_ar]
            ch = children[pi]
            for d in range(Dd):
                cd = ch[:, d]
                valid = cd >= 0
                fwd = (cd >= L) & ((cd - L) >= pi)
                cslot = np.where(
                    cd < L, np.clip(cd, 0, None),
                    L + np.where(cd >= L, first_slot[np.clip(cd - L, 0, P - 1)], 0),
                )
                cslot = np.where(valid & ~fwd, cslot, 0)
                liall[:, gb * D + d] = rows_of(cslot)
                lmall[:, gb * D + d] = (valid & ~fwd).astype(np.float32)
                zcall[:, gb] += (valid & fwd) * np.float32(np.exp(0.5))
        levels.append(dict(nb=nb, pb0=pb0))
        pb0 += nb

    # scan gather calls: per level/sub-batch, <=GCALL-idx segments
    lidx_segs = []          # wrapped [128, n//16] blocks, concatenated later
    scan_calls = []         # per level: per sub-batch: list of (seg_col0, ncols, nidx)
    seg_col0 = 0
    for lv in levels:
        nb = lv["nb"]
        lv_calls = []
        sb0 = 0
        for sbn in _split(nb, SBMAX):
            c_lo = (lv["pb0"] + sb0) * D
            nc_s = sbn * D
            calls = []
            q0 = 0
            while q0 < nc_s:
                qn = min(GCALL // 128, nc_s - q0)        # columns this call
                nidx = qn * 128
                lin = liall[:, c_lo + q0:c_lo + q0 + qn].T.reshape(-1).astype(np.int16)
                lidx_segs.append(_wrap16(lin))
                calls.append((seg_col0, qn, nidx))
                seg_col0 += nidx // 16
                q0 += qn
            lv_calls.append(calls)
            sb0 += sbn
        scan_calls.append(lv_calls)
    lidx16 = np.concatenate(lidx_segs, axis=1)

    return dict(
        L=L, P=P, NEFF=NEFF, NBT=NBT, NBC=NBC, LB=LB, LBC=LBC, Kw=Kw,
        gidx4=gidx4, mw4=mw4, vocab4=vocab4,
        levels=levels, nlev=nlev,
        lidx16=lidx16, scan_calls=scan_calls,
        lmall=lmall, zcall=zcall, TOTB=TOTB, TOTC=TOTC,
    )


# ---------------------------------------------------------------------------
# device program
# ---------------------------------------------------------------------------

def _build(plan, vocab):
    NBT, NBC, LB, LBC = plan["NBT"], plan["NBC"], plan["LB"], plan["LBC"]
    NEFF = plan["NEFF"]
    TOTB, TOTC = plan["TOTB"], plan["TOTC"]
    NLOC = NBC * 128
    NLEAF = LBC * 128                   # leaf columns per core (1024)
    NPAR = NBC - LBC                    # parent blocks per core
    NP1 = -(-plan["levels"][0]["nb"] // NCORES)   # parent blocks in agP1
    NP1 = min(NP1, NPAR)
    NP2 = NPAR - NP1
    LIDXW = plan["lidx16"].shape[1]
    levels = plan["levels"]
    scan_calls = plan["scan_calls"]

    nc = bacc.Bacc(None, num_devices=NCORES, dynamic_dma_scratch_size=SCRATCH)

    Kw = plan["Kw"]
    vocab4 = plan["vocab4"]
    e4 = nc.dram_tensor("e4", [vocab4, 4 * H], BF16, kind="ExternalInput")
    gx4_e = nc.dram_tensor("gx4_e", [128, NBC * Kw * 8], I16, kind="ExternalInput")
    mw4_e = nc.dram_tensor("mw4_e", [128, NBC * Kw * 4], BF16, kind="ExternalInput")
    wz_e = nc.dram_tensor("wz_e", [H, H], BF16, kind="ExternalInput")
    wr_e = nc.dram_tensor("wr_e", [H, H], BF16, kind="ExternalInput")
    wh_e = nc.dram_tensor("wh_e", [H, H], BF16, kind="ExternalInput")
    wa_e = nc.dram_tensor("wa_e", [H, H], BF16, kind="ExternalInput")
    uz_t = nc.dram_tensor("uz_t", [H, H], BF16, kind="ExternalInput")
    ur_t = nc.dram_tensor("ur_t", [H, H], BF16, kind="ExternalInput")
    uh_t = nc.dram_tensor("uh_t", [H, H], BF16, kind="ExternalInput")
    bias_e = nc.dram_tensor("bias_e", [H, 3], F32, kind="ExternalInput")
    wo_e = nc.dram_tensor("wo_e", [H + 1, NCLASS], F32, kind="ExternalInput")
    li_e = nc.dram_tensor("li_e", [128, LIDXW], I16, kind="ExternalInput")
    lm_e = nc.dram_tensor("lm_e", [128, TOTC], F32, kind="ExternalInput")
    zc_e = nc.dram_tensor("zc_e", [128, TOTB], F32, kind="ExternalInput")

    out_t = nc.dram_tensor("out", [1, NCLASS], F32, kind="ExternalOutput")

    # node_h rows stored bf16, duplicated ([h, h] = 256B records) so the
    # dma_gather 256B-min element covers exactly one node
    node_h = nc.dram_tensor("node_h", [NEFF, 2 * H], BF16)               # Internal
    # leaf h travels ROW-major (owner pre-transposes its 8 blocks)
    agL_in = nc.dram_tensor("agL_in", [128, LBC * H], BF16)              # Internal
    agP1_in = nc.dram_tensor("agP1_in", [H, NP1 * 128], BF16)            # Internal
    agP2_in = nc.dram_tensor("agP2_in", [H, NP2 * 128], BF16)            # Internal
    agL_out = nc.dram_tensor("agL_out", [NCORES, 128, LBC * H], BF16,
                             addr_space="Shared")
    agP1_out = nc.dram_tensor("agP1_out", [NCORES, H, NP1 * 128], BF16,
                              addr_space="Shared")
    agP2_out = nc.dram_tensor("agP2_out", [NCORES, H, NP2 * 128], BF16,
                              addr_space="Shared")

    node_h3 = node_h[:].rearrange("(p b) e -> p b e", p=128)             # [128,NBT,128]

    with tile.TileContext(nc) as tc:
        with (
            tc.tile_pool(name="const", bufs=1) as constp,
            tc.tile_pool(name="small", bufs=2) as small,
            tc.tile_pool(name="lvl", bufs=2) as lvlp,
            tc.tile_pool(name="lvl1", bufs=1) as lvl1,
            tc.tile_pool(name="lvl3", bufs=3) as lvl3,
            tc.tile_pool(name="ph1", bufs=2) as ph1p,
            tc.tile_pool(name="scan", bufs=2) as scanp,
            tc.tile_pool(name="psum", bufs=2, space="PSUM") as psum,
            tc.tile_pool(name="psumt", bufs=2, space="PSUM") as psumt,
            tc.tile_pool(name="psumq", bufs=1, space="PSUM") as psumq,
        ):
            ident = constp.tile([128, 128], F32, tag="ident")
            make_identity(nc, ident[:])
            identb = constp.tile([128, 128], BF16, tag="identb")
            nc.vector.tensor_copy(out=identb[:], in_=ident[:])

            wts = {}
            for nm, t in [("wz", wz_e), ("wr", wr_e), ("wh", wh_e), ("wa", wa_e),
                          ("uz", uz_t), ("ur", ur_t), ("uh", uh_t)]:
                w = constp.tile([H, H], BF16, tag=f"w_{nm}")
                nc.sync.dma_start(out=w[:], in_=t[:])
                wts[nm] = w
            bias = constp.tile([H, 3], F32, tag="bias")
            nc.sync.dma_start(out=bias[:], in_=bias_e[:])
            bz, br, bh = bias[:, 0:1], bias[:, 1:2], bias[:, 2:3]
            wo = constp.tile([H + 1, NCLASS], F32, tag="w_wo")
            nc.sync.dma_start(out=wo[:], in_=wo_e[:])

            liall = constp.tile([128, LIDXW], I16, tag="liall")
            nc.sync.dma_start(out=liall[:], in_=li_e[:])
            lmall = constp.tile([128, TOTC], F32, tag="lmall")
            nc.sync.dma_start(out=lmall[:], in_=lm_e[:])
            zcall = constp.tile([128, TOTB], F32, tag="zcall")
            nc.sync.dma_start(out=zcall[:], in_=zc_e[:])

            gx4_t = constp.tile([128, NBC * Kw * 8], I16, tag="gx4")
            nc.sync.dma_start(out=gx4_t[:], in_=gx4_e[:])
            mw4_t = constp.tile([128, NBC * Kw * 4], BF16, tag="mw4")
            nc.sync.dma_start(out=mw4_t[:], in_=mw4_e[:])

            xeT_loc = constp.tile([H, NLOC], BF16, tag="xeT_loc")

            # zero-init the parent region of node_h: scan gathers read the
            # whole tensor AP, so unwritten rows must be finite
            zt = constp.tile([128, 9 * 2 * H], BF16, tag="zeros")
            nc.vector.memset(zt[:], 0.0)
            zb0 = LB
            while zb0 < NBT:
                zn = min(9, NBT - zb0)
                nc.sync.dma_start(out=node_h3[:, zb0:zb0 + zn, :],
                                  in_=zt[:, :zn * 2 * H])
                zb0 += zn

            # shared num_idxs registers for the dma_gather calls
            greg = {}

            def nreg(n):
                if n not in greg:
                    greg[n] = nc.gpsimd.to_reg(n)
                return greg[n]

            # ---------------- phase 1: embedding ---------------------------
            def emb_chunk(m):
                """local block m: 4-row-packed bf16 gather (512B records)
                straight into gm, the 1-of-4 select folded into the mw
                mask (expanded over H on the idle Activation engine so the
                multiply runs in DVE 2x mode), bf16 tree-reduce, transpose.
                """
                gm = small.tile([128, Kw * 4 * H], BF16, tag="gm")
                mwx = lvl1.tile([128, Kw * 4 * H], BF16, tag="mwx")
                for hf in range(2):
                    nc.scalar.activation(
                        out=mwx[:, hf * Kw * 2 * H:(hf + 1) * Kw * 2 * H]
                        .rearrange("p (s e) -> p s e", e=H),
                        in_=_ap(mw4_t[:, m * Kw * 4 + hf * Kw * 2:
                                      m * Kw * 4 + (hf + 1) * Kw * 2],
                                [(1, Kw * 2), (0, H)]),
                        func=ACT.Copy)
                for q4 in range(4):
                    nc.gpsimd.dma_gather(
                        out_ap=gm[:, q4 * Kw * H:(q4 + 1) * Kw * H]
                        .rearrange("p (j e) -> p j e", e=4 * H),
                        in_ap=e4[:],
                        idxs_ap=gx4_t[:, (m * Kw + q4 * Kw // 4) * 8:
                                      (m * Kw + (q4 + 1) * Kw // 4) * 8],
                        num_idxs=Kw * 32, num_idxs_reg=nreg(Kw * 32),
                        elem_size=4 * H)
                nc.vector.tensor_tensor(
                    out=gm[:, :Kw * 4 * H],
                    in0=gm[:, :Kw * 4 * H],
                    in1=mwx[:, :Kw * 4 * H],
                    op=ALU.mult)
                hw = Kw * 2
                while hw >= 1:
                    nc.vector.tensor_tensor(
                        out=gm[:, :hw * H], in0=gm[:, :hw * H],
                        in1=gm[:, hw * H:2 * hw * H], op=ALU.add)
                    hw //= 2
                pt = psumt.tile([H, 128], BF16, tag="pt")
                nc.tensor.transpose(out=pt[:], in_=gm[:, :H], identity=identb[:])
                # copy via DVE, not ACT: an ACT-queue copy would make later
                # mask expansions wait on this block's tree
                nc.vector.tensor_copy(
                    out=xeT_loc[:, m * 128:(m + 1) * 128], in_=pt[:])

            for m in range(LBC):                    # leaf blocks first
                emb_chunk(m)

            # ---------------- phase 2a: own-leaf GRU (pre-collective) ------
            lrow = constp.tile([128, LBC * H], BF16, tag="lrow")
            for c0 in range(0, NLEAF, 512):
                sl = slice(c0, c0 + 512)
                pz = psum.tile([H, 512], F32, tag="pz")
                nc.tensor.matmul(out=pz[:], lhsT=wts["wz"][:], rhs=xeT_loc[:, sl],
                                 start=True, stop=True)
                zl = small.tile([H, 512], F32, tag="zt_")
                nc.scalar.activation(out=zl[:], in_=pz[:], func=ACT.Sigmoid, bias=bz)
                ph = psum.tile([H, 512], F32, tag="pz")
                nc.tensor.matmul(out=ph[:], lhsT=wts["wh"][:], rhs=xeT_loc[:, sl],
                                 start=True, stop=True)
                cl = small.tile([H, 512], F32, tag="ct_")
                nc.scalar.activation(out=cl[:], in_=ph[:], func=ACT.Tanh, bias=bh)
                nc.vector.tensor_tensor(out=zl[:], in0=zl[:], in1=cl[:], op=ALU.mult)
                nc.vector.tensor_tensor(out=cl[:], in0=cl[:], in1=zl[:], op=ALU.subtract)
                # transpose own leaf h to ROW layout for the collective
                for gr in range(4):
                    pt = psumt.tile([128, H], F32, tag="pt")
                    nc.tensor.transpose(
                        out=pt[:], in_=cl[:, gr * 128:(gr + 1) * 128],
                        identity=ident[:H, :H])
                    nc.scalar.activation(
                        out=lrow[:, (c0 // 128 + gr) * H:(c0 // 128 + gr + 1) * H],
                        in_=pt[:], func=ACT.Copy)

            nc.sync.dma_start(out=agL_in[:], in_=lrow[:])

            for m in range(LBC, LBC + NP1):         # parent blocks, 1st half
                emb_chunk(m)
                nc.sync.dma_start(
                    out=agP1_in[:, (m - LBC) * 128:(m - LBC + 1) * 128],
                    in_=xeT_loc[:, m * 128:(m + 1) * 128])

            xeT = constp.tile([H, TOTB * 128], BF16, tag="xeT")

            finacc = constp.tile([H, 1], F32, tag="finacc")
            nc.vector.memset(finacc[:], -1e30)
            maxT = constp.tile([H, SBMAX * 128], BF16, tag="maxT")
            nc.vector.memset(maxT[:], -1e30)
            # output-head constants prepared off the critical tail
            fin = small.tile([H + 1, 1], F32, tag="fin")
            nc.vector.memset(fin[H:H + 1, :], 1.0)

            def sub_gather(l, si):
                """Emit the child-row gather calls for (level, sub-batch)."""
                g = scanp.tile([128, SBMAX * D * 2 * H], BF16, tag="sg")
                col = 0
                for (seg0, qn, nidx) in scan_calls[l][si]:
                    nc.gpsimd.dma_gather(
                        out_ap=g[:, col * 2 * H:(col + qn) * 2 * H].rearrange(
                            "p (j e) -> p j e", e=2 * H),
                        in_ap=node_h[:],
                        idxs_ap=liall[:, seg0:seg0 + nidx // 16],
                        num_idxs=nidx, num_idxs_reg=nreg(nidx), elem_size=2 * H)
                    col += qn
                return g

            # level-0 gathers depend only on leaf rows: issue the first two
            # now so they run under the collectives (2 tile bufs — more
            # would recycle a buffer before its reader is even emitted)
            for m in range(LBC + NP1, NBC):         # parent blocks, 2nd half
                emb_chunk(m)
                nc.sync.dma_start(
                    out=agP2_in[:, (m - LBC - NP1) * 128:(m - LBC - NP1 + 1) * 128],
                    in_=xeT_loc[:, m * 128:(m + 1) * 128])

            # collectives are issued only after every embedding gather has
            # been dispatched (the Pool SEQ blocks in-order on the ag*_in
            # data waits); the in-buffer copies travel on the SP queue,
            # emitted right after their producers.  Consumers of each
            # collective's output are emitted directly after it.
            nc.gpsimd.collective_compute(
                "AllGather", ALU.bypass,
                replica_groups=[list(range(NCORES))],
                ins=[agL_in[:].opt()], outs=[agL_out[:].opt()])
            # leaf rows to node_h: dup-half writes
            for c in range(NCORES):
                for hf in range(2):
                    nc.sync.dma_start(
                        out=_ap(node_h3[:, c:c + 1, hf * H:hf * H + 1],
                                [(NCORES * 2 * H, LBC), (1, H)]),
                        in_=agL_out[c].rearrange("p (j e) -> p j e", e=H))

            nc.gpsimd.collective_compute(
                "AllGather", ALU.bypass,
                replica_groups=[list(range(NCORES))],
                ins=[agP1_in[:].opt()], outs=[agP1_out[:].opt()])

            # parent xe^T assembly, first half: core c's slab covers parent
            # blocks c + 8j', j'=0..NP1-1 -> xeT cols c*128 + j'*1024
            for c in range(NCORES):
                nc.sync.dma_start(
                    out=_ap(xeT[:, c * 128:c * 128 + 1],
                            [(NCORES * 128, NP1), (1, 128)]),
                    in_=agP1_out[c].rearrange("e (j x) -> e j x", x=128))

            NL0 = len(scan_calls[0])
            l0_g = [sub_gather(0, si) for si in range(min(2, NL0))]

            nc.gpsimd.collective_compute(
                "AllGather", ALU.bypass,
                replica_groups=[list(range(NCORES))],
                ins=[agP2_in[:].opt()], outs=[agP2_out[:].opt()])

            for c in range(NCORES):
                nc.sync.dma_start(
                    out=_ap(xeT[:, (NCORES * NP1 + c) * 128:
                                 (NCORES * NP1 + c) * 128 + 1],
                            [(NCORES * 128, NP2), (1, 128)]),
                    in_=agP2_out[c].rearrange("e (j x) -> e j x", x=128))

            # ---------------- phase 3: level scan (sub-batched) ------------
            for l, lv in enumerate(levels):
                nb = lv["nb"]
                last_lv = l == len(levels) - 1
                # all of this level's gathers upfront: they only depend on
                # earlier levels' writes, and emitting them before any of
                # this level's writes avoids false whole-tensor deps
                # emit up to 2 gathers upfront (tile pool depth); further
                # sub-batches' gathers are emitted lazily after the first
                # sub-batch's reads exist, so buffer recycling is safe
                gs = l0_g if l == 0 else [
                    sub_gather(l, si)
                    for si in range(min(2, len(scan_calls[l])))]
                # software-pipelined emission: all attention front-ends
                # first, then all gate/write back-ends
                fes = []
                sb0 = 0
                for si, sbn in enumerate(_split(nb, SBMAX)):
                    n_s = sbn * 128
                    nc_s = sbn * D
                    gb0 = lv["pb0"] + sb0
                    c_lo = gb0 * D
                    rhs_s = xeT[:, gb0 * 128:gb0 * 128 + n_s]

                    g = gs[si] if si < len(gs) else sub_gather(l, si)
                    # child h = first half of each 256B dup record
                    g4 = _ap(g[:, 0:1], [(D * 2 * H, sbn), (2 * H, D), (1, H)])

                    # attention query rows straight into PSUM, then to SBUF
                    # bf16 so the mults run in DVE 2x mode
                    qps = psumq.tile([128, SBMAX * H], F32, tag="qps")
                    for b in range(sbn):
                        nc.tensor.matmul(
                            out=qps[:, b * H:(b + 1) * H],
                            lhsT=rhs_s[:, b * 128:(b + 1) * 128],
                            rhs=wts["wa"][:], start=True, stop=True)
                    # logits = sum_e q*child : mult then bf16 tree over H
                    prod = scanp.tile([128, SBMAX * D * H], BF16, tag="sp")
                    pv = prod[:, :nc_s * H]
                    q_ap = _ap(qps[:, 0:1], [(H, sbn), (0, D), (1, H)])
                    nc.vector.tensor_tensor(
                        out=pv.rearrange("p (b d e) -> p b d e", b=sbn, d=D),
                        in0=g4, in1=q_ap, op=ALU.mult)
                    hw = H // 2
                    while hw >= 2:
                        nc.vector.tensor_tensor(
                            out=_ap(prod[:, 0:1], [(H, nc_s), (1, hw)]),
                            in0=_ap(prod[:, 0:1], [(H, nc_s), (1, hw)]),
                            in1=_ap(prod[:, hw:hw + 1], [(H, nc_s), (1, hw)]),
                            op=ALU.add)
                        hw //= 2
                    logit = lvl1.tile([128, SBMAX * D], F32, tag="logit")
                    nc.vector.tensor_tensor(
                        out=logit[:, :nc_s],
                        in0=_ap(prod[:, 0:1], [(H, nc_s), (1, 1)]),
                        in1=_ap(prod[:, 1:2], [(H, nc_s), (1, 1)]),
                        op=ALU.add)
                    lg = logit[:, :nc_s]
                    nc.scalar.activation(out=lg, in_=lg, func=ACT.Sigmoid)
                    # exp(s) on [0,1]: Horner via scalar_tensor_tensor
                    # x = c4*s; x = (x+c3)*s; ...; ev = (x+c0)*mask
                    ex = lvl1.tile([128, SBMAX * D], F32, tag="ex")
                    ev = ex[:, :nc_s]
                    nc.vector.tensor_scalar_mul(out=ev, in0=lg,
                                                scalar1=float(_EXPC[4]))
                    for cf in (_EXPC[3], _EXPC[2], _EXPC[1]):
                        nc.vector.scalar_tensor_tensor(
                            out=ev, in0=ev, scalar=float(cf), in1=lg,
                            op0=ALU.add, op1=ALU.mult)
                    nc.vector.scalar_tensor_tensor(
                        out=ev, in0=ev, scalar=float(_EXPC[0]),
                        in1=lmall[:, c_lo:c_lo + nc_s],
                        op0=ALU.add, op1=ALU.mult)
                    den = lvl1.tile([128, SBMAX], F32, tag="den")
                    nc.vector.tensor_reduce(
                        out=den[:, :sbn],
                        in_=ev.rearrange("p (b d) -> p b d", d=D),
                        axis=AXL.X, op=ALU.add)
                    if plan["zcall"][:, gb0:gb0 + sbn].any():
                        nc.vector.tensor_tensor(
                            out=den[:, :sbn], in0=den[:, :sbn],
                            in1=zcall[:, gb0:gb0 + sbn], op=ALU.add)
                    nc.vector.reciprocal(out=den[:, :sbn], in_=den[:, :sbn])
                    nc.vector.tensor_tensor(
                        out=ev.rearrange("p (b d) -> p b d", d=D),
                        in0=ev.rearrange("p (b d) -> p b d", d=D),
                        in1=_ap(den[:, :sbn], [(1, sbn), (0, D)]), op=ALU.mult)
                    # h_tilde = sum_d attn*child : mult + 2-step bf16 tree
                    nc.vector.tensor_tensor(
                        out=pv.rearrange("p (b d e) -> p b d e", b=sbn, d=D),
                        in0=g4, in1=_ap(ex[:, 0:1], [(D, sbn), (1, D), (0, H)]),
                        op=ALU.mult)
                    nc.vector.tensor_tensor(
                        out=_ap(prod[:, 0:1], [(D * H, sbn), (1, 2 * H)]),
                        in0=_ap(prod[:, 0:1], [(D * H, sbn), (1, 2 * H)]),
                        in1=_ap(prod[:, 2 * H:2 * H + 1], [(D * H, sbn), (1, 2 * H)]),
                        op=ALU.add)
                    ht = lvl3.tile([128, SBMAX * H], BF16, tag="ht")
                    nc.vector.tensor_tensor(
                        out=ht[:, :sbn * H].rearrange("p (b e) -> p b e", e=H),
                        in0=_ap(prod[:, 0:1], [(D * H, sbn), (1, H)]),
                        in1=_ap(prod[:, H:H + 1], [(D * H, sbn), (1, H)]),
                        op=ALU.add)
                    fes.append((sbn, sb0, n_s, rhs_s, ht))
                    sb0 += sbn

                for (sbn, sb0, n_s, rhs_s, ht) in fes:
                    # transpose h_tilde -> [64, n_s] (bf16 for the U matmuls)
                    htT = lvlp.tile([H, SBMAX * 128], BF16, tag="htT")
                    for b in range(sbn):
                        pt = psumt.tile([H, 128], BF16, tag="pt")
                        nc.tensor.transpose(out=pt[:], in_=ht[:, b * H:(b + 1) * H],
                                            identity=identb[:])
                        nc.scalar.activation(out=htT[:, b * 128:(b + 1) * 128],
                                             in_=pt[:], func=ACT.Copy)

                    # gates; weight-grouped matmul passes, one PSUM tile per
                    # 1024-half, activations fused with bias
                    hTt = lvlp.tile([H, SBMAX * 128], BF16, tag="hTt")
                    rh = lvlp.tile([H, SBMAX * 128], BF16, tag="rh")
                    chunks = [(c0, min(512, n_s - c0))
                              for c0 in range(0, n_s, 512)]

                    def gate_mm(wname, uname, urhs, act, bias_ap, dst):
                        for h0 in range(0, n_s, 1024):
                            hw_ = min(1024, n_s - h0)
                            hch = [(c0, cw) for (c0, cw) in chunks
                                   if h0 <= c0 < h0 + hw_]
                            p = psum.tile([H, 1024], F32, tag="pz")
                            for (c0, cw) in hch:
                                nc.tensor.matmul(out=p[:, c0 - h0:c0 - h0 + cw],
                                                 lhsT=wts[wname][:],
                                                 rhs=rhs_s[:, c0:c0 + cw],
                                                 start=True, stop=False)
                            for (c0, cw) in hch:
                                nc.tensor.matmul(out=p[:, c0 - h0:c0 - h0 + cw],
                                                 lhsT=wts[uname][:],
                                                 rhs=urhs[:, c0:c0 + cw],
                                                 start=False, stop=True)
                            nc.scalar.activation(out=dst[:, h0:h0 + hw_],
                                                 in_=p[:, :hw_],
                                                 func=act, bias=bias_ap)

                    # r first: its act -> rh -> c-U chain is the serial tail
                    rT = lvlp.tile([H, SBMAX * 128], BF16, tag="rT")
                    gate_mm("wr", "ur", htT, ACT.Sigmoid, br, rT)
                    nc.vector.tensor_tensor(out=rh[:, :n_s], in0=rT[:, :n_s],
                                            in1=htT[:, :n_s], op=ALU.mult)
                    zT = lvlp.tile([H, SBMAX * 128], BF16, tag="zT")
                    gate_mm("wz", "uz", htT, ACT.Sigmoid, bz, zT)
                    cT = lvlp.tile([H, SBMAX * 128], BF16, tag="cT")
                    gate_mm("wh", "uh", rh, ACT.Tanh, bh, cT)
                    # h = c + z*(ht - c)
                    nc.vector.tensor_tensor(out=hTt[:, :n_s], in0=htT[:, :n_s],
                                            in1=cT[:, :n_s], op=ALU.subtract)
                    nc.vector.tensor_tensor(out=hTt[:, :n_s], in0=hTt[:, :n_s],
                                            in1=zT[:, :n_s], op=ALU.mult)
                    nc.vector.tensor_tensor(out=hTt[:, :n_s], in0=hTt[:, :n_s],
                                            in1=cT[:, :n_s], op=ALU.add)
                    # transpose back into a dup row buffer, then write this
                    # sub-batch's rows with a single DMA
                    if not last_lv:
                        hrow = lvlp.tile([128, SBMAX * H], BF16, tag="hrow")
                        for b in range(sbn):
                            pt = psumt.tile([128, H], BF16, tag="pt")
                            nc.tensor.transpose(out=pt[:],
                                                in_=hTt[:, b * 128:(b + 1) * 128],
                                                identity=identb[:H, :H])
                            nc.scalar.activation(
                                out=hrow[:, b * H:(b + 1) * H],
                                in_=pt[:], func=ACT.Copy)
                        bb = LB + lv["pb0"] + sb0
                        for hf in range(2):
                            nc.sync.dma_start(
                                out=_ap(node_h3[:, bb:bb + 1, hf * H:hf * H + 1],
                                        [(2 * H, sbn), (1, H)]),
                                in_=_ap(hrow[:, 0:1], [(H, sbn), (1, H)]))
                    # fold this sub-batch into the running parent-h max
                    nc.vector.tensor_tensor(out=maxT[:, :n_s], in0=maxT[:, :n_s],
                                            in1=hTt[:, :n_s], op=ALU.max)

            # prefetch the Exp activation table while the tail of the scan
            # still runs (the head's softmax otherwise pays the swap serially)
            nc.scalar.activation(out=fin[H:H + 1, :], in_=fin[H:H + 1, :],
                                 func=ACT.Exp)
            nc.vector.memset(fin[H:H + 1, :], 1.0)

            # ---------------- phase 4: output head -------------------------
            nc.vector.tensor_reduce(out=finacc[:], in_=maxT[:],
                                    axis=AXL.X, op=ALU.max)
            nc.vector.tensor_copy(out=fin[:H, :], in_=finacc[:])
            po = psumt.tile([NCLASS, 1], F32, tag="pt")
            nc.tensor.matmul(out=po[:], lhsT=wo[:], rhs=fin[:], start=True, stop=True)
            s4 = small.tile([NCLASS, 1], F32, tag="s4")
            nc.vector.tensor_copy(out=s4[:], in_=po[:])
            s4t = small.tile([1, NCLASS], F32, tag="s4t")
            nc.sync.dma_start(out=s4t[:], in_=s4[:])
            nc.scalar.activation(out=s4t[:], in_=s4t[:], func=ACT.Exp)
            ssum = small.tile([1, 1], F32, tag="ssum")
            nc.vector.tensor_reduce(out=ssum[:], in_=s4t[:], axis=AXL.X, op=ALU.add)
            nc.vector.reciprocal(out=ssum[:], in_=ssum[:])
            nc.vector.tensor_tensor(out=s4t[:], in0=s4t[:],
                                    in1=_ap(ssum[:], [(0, NCLASS)]), op=ALU.mult)
            nc.sync.dma_start(out=out_t[:], in_=s4t[:])

    nc.compile()
    return nc


# ---------------------------------------------------------------------------
# entry point
# ---------------------------------------------------------------------------

def _to_bf16(x):
    import ml_dtypes
    return np.asarray(x, np.float32).astype(ml_dtypes.bfloat16)


def _prepare(inputs):
    x_word = np.asarray(inputs["x_word"], np.float32)
    x_index = np.asarray(inputs["x_index"], np.int32)
    tree = np.asarray(inputs["tree"], np.int32)
    E = np.asarray(inputs["E_bu"], np.float32)
    vocab = E.shape[1]

    plan = _plan(x_word, x_index, tree, vocab)
    nc = _build(plan, vocab)

    bias3 = np.stack([
        np.asarray(inputs["b_z_bu"], np.float32).reshape(-1),
        np.asarray(inputs["b_r_bu"], np.float32).reshape(-1),
        np.asarray(inputs["b_h_bu"], np.float32).reshape(-1),
    ], axis=1)

    vocab4 = plan["vocab4"]
    e4 = np.ascontiguousarray(E.T).reshape(vocab4, 4 * H)
    shared = {
        "e4": _to_bf16(e4),
        "wz_e": _to_bf16(np.asarray(inputs["W_z_bu"]).T),
        "wr_e": _to_bf16(np.asarray(inputs["W_r_bu"]).T),
        "wh_e": _to_bf16(np.asarray(inputs["W_h_bu"]).T),
        "wa_e": _to_bf16(inputs["W_attn"]),
        "uz_t": _to_bf16(np.asarray(inputs["U_z_bu"]).T),
        "ur_t": _to_bf16(np.asarray(inputs["U_r_bu"]).T),
        "uh_t": _to_bf16(np.asarray(inputs["U_h_bu"]).T),
        "bias_e": bias3,
        "wo_e": np.vstack([np.asarray(inputs["W_out_bu"], np.float32).T,
                           np.asarray(inputs["b_out_bu"], np.float32).reshape(1, -1)]),
        "li_e": plan["lidx16"],
        "lm_e": plan["lmall"],
        "zc_e": plan["zcall"],
    }

    in_maps = []
    for c in range(NCORES):
        m = dict(shared)
        m["gx4_e"] = plan["gidx4"][c]
        m["mw4_e"] = _to_bf16(plan["mw4"][c])
        in_maps.append(m)
    return nc, in_maps


def kernel(**inputs) -> np.ndarray:
    from concourse.bass_utils import run_bass_kernel_spmd
    nc, in_maps = _prepare(inputs)
    res = run_bass_kernel_spmd(nc, in_maps, core_ids=list(range(NCORES)))
    return res.results[0]["out"].reshape(NCLASS).astype(np.float32)
